# revision 15
# baseline (speedup 1.0000x reference)
"""Trainium2 Bass kernel for ClassCenterCalculator (segment_reduce).

reference:
    predicted = argmax(pseudo_labels, axis=1)            # [B]
    sums    = segment_sum(features, predicted, C)        # [C, D]
    counts  = segment_sum(ones(B), predicted, C)         # [C]
    centers = where(counts>0, sums/max(counts,1), sums)  # [C, D]

Strategy (data-parallel over 8 NeuronCores):
  - shard batch dim: each core gets B/8 = 32768 rows of features/labels
  - on-core: one-hot(argmax(labels)) via DVE compares, then
    sums = one_hot.T @ features via PE matmul accumulated in PSUM
    (contraction over the batch dim, 128 rows per matmul),
    counts = ones.T @ partial-count reduction (one tiny matmul)
  - each core writes a [3, 513] partial (sums ++ counts); host adds the
    8 tiny partials and normalizes.
"""

import os
import sys

for _p in ("/root/.axon_site/_ro/trn_rl_repo", "/opt/trn_rl_repo"):
    if os.path.isdir(_p) and _p not in sys.path:
        sys.path.append(_p)

import numpy as np

import concourse.bacc as bacc
import concourse.mybir as mybir
import concourse.tile as tile
from concourse.alu_op_type import AluOpType
from concourse.bass_utils import run_bass_kernel_spmd

B = 262144
D = 512
C = 3
NCORES = 8
BS = B // NCORES          # rows per core = 32768
P = 128                   # partitions / matmul contraction tile
NCH = BS // P             # 128-row chunks per core = 256
CH_PER_DMA = 8            # feature chunks per DMA  -> 2 MiB transfers
NDMA = NCH // CH_PER_DMA  # feature DMAs per core = 32

F32 = mybir.dt.float32
BF16 = mybir.dt.bfloat16  # PE-facing dtype: 1 cycle/row matmul (fp32 is 4)

_CACHE = {}
LAST_RESULT = None


def _build():
    nc = bacc.Bacc("TRN2", target_bir_lowering=False)

    feat = nc.declare_dram_parameter("features", [BS, D], F32, isOutput=False)
    labs = nc.declare_dram_parameter("labels", [BS, C], F32, isOutput=False)
    out = nc.declare_dram_parameter("out", [C, D + 1], F32, isOutput=True)

    with tile.TileContext(nc) as tc:
        with (
            tc.tile_pool(name="persist", bufs=1) as pp,
            tc.tile_pool(name="feats", bufs=6) as fp,
            tc.tile_pool(name="featsb", bufs=6) as fb,
            tc.tile_pool(name="psum", bufs=1, space="PSUM") as psp,
        ):
            # ---- labels: [BS, 3] -> SBUF [128, NCH, 3] in NATURAL row-block
            # layout: (p, q, k) = labels[256*p + q, k].  3 KiB contiguous per
            # partition -> 128 DMA descriptors (vs 32768 for row%128 layout).
            # Segment-sum is row-order invariant, so features chunks below use
            # the matching strided AP (row = 256*p + m) instead of row%128.
            lab = pp.tile([P, NCH, C], F32)
            # scalar-engine HWDGE ring: keeps the sync ring free for features
            nc.scalar.dma_start(lab[:], labs.rearrange("(p q) k -> p q k", p=P))

            # ---- PE warmup: ~6us of dummy matmuls during the initial DMA
            # fill so the HAM clock gate reaches K=8/8 before real work.
            warm = pp.tile([P, D], BF16)
            nc.gpsimd.memset(warm[:], 0.0)
            psum_w = psp.tile([C, D], F32)
            for _ in range(16):
                nc.tensor.matmul(psum_w[:], warm[:, :C], warm[:], start=True, stop=True)

            # ---- one-hot of argmax (first-max-wins, matches jnp.argmax ties)
            # values are 0/1 -> exact in bf16
            oh = pp.tile([P, NCH, C], BF16)
            t0 = pp.tile([P, NCH], F32)
            t1 = pp.tile([P, NCH], F32)
            l0, l1, l2 = (lab[:, :, k] for k in range(C))
            # oh0 = (l0>=l1)&(l0>=l2); oh1 = (l1>l0)&(l1>=l2); oh2 = (l2>l0)&(l2>l1)
            nc.vector.tensor_tensor(t0[:], l0, l1, AluOpType.is_ge)
            nc.vector.tensor_tensor(t1[:], l0, l2, AluOpType.is_ge)
            nc.vector.tensor_tensor(oh[:, :, 0], t0[:], t1[:], AluOpType.mult)
            nc.vector.tensor_tensor(t0[:], l1, l0, AluOpType.is_gt)
            nc.vector.tensor_tensor(t1[:], l1, l2, AluOpType.is_ge)
            nc.vector.tensor_tensor(oh[:, :, 1], t0[:], t1[:], AluOpType.mult)
            nc.vector.tensor_tensor(t0[:], l2, l0, AluOpType.is_gt)
            nc.vector.tensor_tensor(t1[:], l2, l1, AluOpType.is_gt)
            nc.vector.tensor_tensor(oh[:, :, 2], t0[:], t1[:], AluOpType.mult)

            # ---- counts: per-partition partial then 1-col matmul with ones
            cntp = pp.tile([P, C], F32)
            nc.vector.tensor_reduce(
                cntp[:], oh[:].rearrange("p j k -> p k j"),
                axis=mybir.AxisListType.X, op=AluOpType.add,
            )
            ones = pp.tile([P, 1], F32)
            nc.gpsimd.memset(ones[:], 1.0)
            psum_c = psp.tile([C, 1], F32)
            nc.tensor.matmul(psum_c[:], cntp[:], ones[:], start=True, stop=True)  # exact fp32: counts are small ints

            # ---- segment sums: 256 accumulated matmuls, K-tiled over batch
            psum_s = psp.tile([C, D], F32)
            feat_blk = feat.rearrange("(p q) d -> p q d", p=P)  # row = 256*p + q
            for g in range(NDMA):
                ft = fp.tile([P, CH_PER_DMA, D], F32)
                m0 = g * CH_PER_DMA
                nc.sync.dma_start(ft[:], feat_blk[:, m0:m0 + CH_PER_DMA, :])
                ftb = fb.tile([P, CH_PER_DMA, D], BF16)
                for c2 in range(0, CH_PER_DMA, 2):
                    # 2-chunk cast granularity: matmuls start sooner, PE gaps
                    # stay short (HAM stays warm)
                    nc.vector.tensor_copy(ftb[:, c2:c2 + 2, :], ft[:, c2:c2 + 2, :])
                    for c in (c2, c2 + 1):
                        j = g * CH_PER_DMA + c
                        nc.tensor.matmul(
                            psum_s[:], oh[:, j, :], ftb[:, c, :],
                            start=(j == 0), stop=(j == NCH - 1),
                        )

            # ---- pack [3, 513] partial and store
            res = pp.tile([C, D + 1], F32)
            nc.vector.tensor_copy(res[:, 0:D], psum_s[:])
            nc.vector.tensor_copy(res[:, D:D + 1], psum_c[:])
            nc.sync.dma_start(out[:], res[:])

    nc.compile()
    return nc


def kernel(features: np.ndarray, pseudo_labels: np.ndarray) -> np.ndarray:
    global LAST_RESULT
    if "nc" not in _CACHE:
        _CACHE["nc"] = _build()
    nc = _CACHE["nc"]

    features = np.ascontiguousarray(np.asarray(features, dtype=np.float32))
    labels = np.ascontiguousarray(np.asarray(pseudo_labels, dtype=np.float32))

    in_maps = [
        {
            "features": features[i * BS:(i + 1) * BS],
            "labels": labels[i * BS:(i + 1) * BS],
        }
        for i in range(NCORES)
    ]
    res = run_bass_kernel_spmd(nc, in_maps, core_ids=list(range(NCORES)))
    LAST_RESULT = res

    partial = np.stack([np.asarray(res.results[i]["out"]) for i in range(NCORES)])
    total = partial.sum(axis=0, dtype=np.float32)  # [3, 513]
    sums, counts = total[:, :D], total[:, D]
    centers = np.where(
        (counts > 0)[:, None],
        sums / np.maximum(counts, 1.0)[:, None],
        sums,
    ).astype(np.float32)
    return centers


# revision 18
# speedup vs baseline: 1.2668x; 1.2668x over previous
"""Trainium2 Bass kernel for ClassCenterCalculator (segment_reduce).

reference:
    predicted = argmax(pseudo_labels, axis=1)            # [B]
    sums    = segment_sum(features, predicted, C)        # [C, D]
    counts  = segment_sum(ones(B), predicted, C)         # [C]
    centers = where(counts>0, sums/max(counts,1), sums)  # [C, D]

Strategy (data-parallel over 8 NeuronCores):
  - shard batch dim: each core gets B/8 = 32768 rows of features/labels
  - on-core: one-hot(argmax(labels)) via DVE compares, then
    sums = one_hot.T @ features via PE matmul accumulated in PSUM
    (contraction over the batch dim, 128 rows per matmul),
    counts = ones.T @ partial-count reduction (one tiny matmul)
  - each core writes a [3, 513] partial (sums ++ counts); host adds the
    8 tiny partials and normalizes.
"""

import os
import sys

for _p in ("/root/.axon_site/_ro/trn_rl_repo", "/opt/trn_rl_repo"):
    if os.path.isdir(_p) and _p not in sys.path:
        sys.path.append(_p)

import numpy as np

import concourse.bacc as bacc
import concourse.mybir as mybir
import concourse.tile as tile
from concourse.alu_op_type import AluOpType
from concourse.bass_utils import run_bass_kernel_spmd

B = 262144
D = 512
C = 3
NCORES = 8
BS = B // NCORES          # rows per core = 32768
P = 128                   # partitions / matmul contraction tile
NCH = BS // P             # 128-row chunks per core = 256
# feature-DMA schedule (chunks per DMA, 1 chunk = 128 rows = 256 KiB):
# small transfers first (data starts landing ASAP) and last (short tail),
# 4 MiB transfers in steady state.
DMA_SCHED = [2, 2, 4, 8] + [16] * 14 + [8, 4, 2, 2]
assert sum(DMA_SCHED) == NCH

F32 = mybir.dt.float32
BF16 = mybir.dt.bfloat16  # PE-facing dtype: 1 cycle/row matmul (fp32 is 4)

_CACHE = {}
LAST_RESULT = None


def _build():
    nc = bacc.Bacc("TRN2", target_bir_lowering=False)

    feat = nc.declare_dram_parameter("features", [BS, D], F32, isOutput=False)
    labs = nc.declare_dram_parameter("labels", [BS, C], F32, isOutput=False)
    out = nc.declare_dram_parameter("out", [C, D + 1], F32, isOutput=True)

    with tile.TileContext(nc) as tc:
        with (
            tc.tile_pool(name="persist", bufs=1) as pp,
            tc.tile_pool(name="feats", bufs=3) as fp,
            tc.tile_pool(name="featsb", bufs=3) as fb,
            tc.tile_pool(name="psum", bufs=1, space="PSUM") as psp,
        ):
            # ---- labels: [BS, 3] -> SBUF [128, NCH, 3] in NATURAL row-block
            # layout: (p, q, k) = labels[256*p + q, k].  3 KiB contiguous per
            # partition -> 128 DMA descriptors (vs 32768 for row%128 layout).
            # Segment-sum is row-order invariant, so features chunks below use
            # the matching strided AP (row = 256*p + m) instead of row%128.
            lab = pp.tile([P, NCH, C], F32)
            # scalar-engine HWDGE ring: keeps the sync ring free for features
            nc.scalar.dma_start(lab[:], labs.rearrange("(p q) k -> p q k", p=P))

            # ---- PE warmup: ~6us of dummy matmuls during the initial DMA
            # fill so the HAM clock gate reaches K=8/8 before real work.
            warm = pp.tile([P, D], BF16)
            nc.gpsimd.memset(warm[:], 0.0)
            psum_w = psp.tile([C, D], F32)
            for _ in range(16):
                nc.tensor.matmul(psum_w[:], warm[:, :C], warm[:], start=True, stop=True)

            # ---- one-hot of argmax (first-max-wins, matches jnp.argmax ties)
            # values are 0/1 -> exact in bf16
            oh = pp.tile([P, NCH, C], BF16)
            t0 = pp.tile([P, NCH], F32)
            t1 = pp.tile([P, NCH], F32)
            l0, l1, l2 = (lab[:, :, k] for k in range(C))
            # oh0 = (l0>=l1)&(l0>=l2); oh1 = (l1>l0)&(l1>=l2); oh2 = (l2>l0)&(l2>l1)
            nc.vector.tensor_tensor(t0[:], l0, l1, AluOpType.is_ge)
            nc.vector.tensor_tensor(t1[:], l0, l2, AluOpType.is_ge)
            nc.vector.tensor_tensor(oh[:, :, 0], t0[:], t1[:], AluOpType.mult)
            nc.vector.tensor_tensor(t0[:], l1, l0, AluOpType.is_gt)
            nc.vector.tensor_tensor(t1[:], l1, l2, AluOpType.is_ge)
            nc.vector.tensor_tensor(oh[:, :, 1], t0[:], t1[:], AluOpType.mult)
            nc.vector.tensor_tensor(t0[:], l2, l0, AluOpType.is_gt)
            nc.vector.tensor_tensor(t1[:], l2, l1, AluOpType.is_gt)
            nc.vector.tensor_tensor(oh[:, :, 2], t0[:], t1[:], AluOpType.mult)

            # ---- counts: per-partition partial then 1-col matmul with ones
            cntp = pp.tile([P, C], F32)
            nc.vector.tensor_reduce(
                cntp[:], oh[:].rearrange("p j k -> p k j"),
                axis=mybir.AxisListType.X, op=AluOpType.add,
            )
            ones = pp.tile([P, 1], F32)
            nc.gpsimd.memset(ones[:], 1.0)
            psum_c = psp.tile([C, 1], F32)
            nc.tensor.matmul(psum_c[:], cntp[:], ones[:], start=True, stop=True)  # exact fp32: counts are small ints

            # ---- segment sums: 256 accumulated matmuls, K-tiled over batch
            psum_s = psp.tile([C, D], F32)
            feat_blk = feat.rearrange("(p q) d -> p q d", p=P)  # row = 256*p + q
            m0 = 0
            for ch in DMA_SCHED:
                ft = fp.tile([P, ch, D], F32, tag="ft")
                nc.sync.dma_start(ft[:], feat_blk[:, m0:m0 + ch, :])
                ftb = fb.tile([P, ch, D], BF16, tag="ftb")
                for c2 in range(0, ch, 2):
                    # 2-chunk cast granularity: matmuls start sooner, PE gaps
                    # stay short (HAM stays warm)
                    nc.vector.tensor_copy(ftb[:, c2:c2 + 2, :], ft[:, c2:c2 + 2, :])
                    for c in (c2, c2 + 1):
                        j = m0 + c
                        nc.tensor.matmul(
                            psum_s[:], oh[:, j, :], ftb[:, c, :],
                            start=(j == 0), stop=(j == NCH - 1),
                        )
                m0 += ch

            # ---- pack [3, 513] partial and store
            res = pp.tile([C, D + 1], F32)
            nc.vector.tensor_copy(res[:, 0:D], psum_s[:])
            nc.vector.tensor_copy(res[:, D:D + 1], psum_c[:])
            nc.sync.dma_start(out[:], res[:])

    nc.compile()
    return nc


def kernel(features: np.ndarray, pseudo_labels: np.ndarray) -> np.ndarray:
    global LAST_RESULT
    if "nc" not in _CACHE:
        _CACHE["nc"] = _build()
    nc = _CACHE["nc"]

    features = np.ascontiguousarray(np.asarray(features, dtype=np.float32))
    labels = np.ascontiguousarray(np.asarray(pseudo_labels, dtype=np.float32))

    in_maps = [
        {
            "features": features[i * BS:(i + 1) * BS],
            "labels": labels[i * BS:(i + 1) * BS],
        }
        for i in range(NCORES)
    ]
    res = run_bass_kernel_spmd(nc, in_maps, core_ids=list(range(NCORES)))
    LAST_RESULT = res

    partial = np.stack([np.asarray(res.results[i]["out"]) for i in range(NCORES)])
    total = partial.sum(axis=0, dtype=np.float32)  # [3, 513]
    sums, counts = total[:, :D], total[:, D]
    centers = np.where(
        (counts > 0)[:, None],
        sums / np.maximum(counts, 1.0)[:, None],
        sums,
    ).astype(np.float32)
    return centers
